# revision 1
# baseline (speedup 1.0000x reference)
"""Trainium2 Bass kernel for the sparse-attention scoring module.

Reference computation (S=2048, B=32, H=1024):
    energy[s,b,:]   = enc[s,b,:] @ W^T + bias            # [S,B,H]
    attn[b,s]       = hidden[b,:] . energy[s,b,:]        # [B,S]
    out             = softmax(attn, axis=1)[None]        # [1,B,S]

Algebraic rewrite:
    attn[b,s] = (hidden[b,:] @ W) . enc[s,b,:]  +  hidden[b,:] . bias
The bias term is constant per row b and cancels in the softmax, so the
kernel never touches `b`.  The [S,B,H] x [H,H] matmul (137 GFLOP)
collapses to a [B,H] x [H,H] matmul plus a batched dot-product, making
the kernel memory-bound on streaming encoder_outputs (268 MB) once.

Sharding: data-parallel over batch.  Each of the 8 cores gets 4 batches:
    enc shard [2048, 4, 1024], hidden shard [4, 1024], full W.

v2 design (production path, build_nc2; ~110 us vs the 140 us v1):
  - prologue: W loaded FIRST and alone on the sync HWDGE ring (~12 us,
    the et stream queues behind it); u = hidden @ W on TensorE, then
    vbcat[p, b*H+h] = u[b,h] replicated to all partitions via selector
    matmuls.
  - stream: 16 et tiles [128, 4*1024] DMA'd at ~428 GB/s (measured; the
    HW dmaonly floor is 78.4 us/pass).  Each (tile, b) pair is ONE DVE
    affine_mul_reduce: product -> throwaway `dump`, accum -> logits
    column.  One pass over the data instead of multiply + separate
    reduce, so DVE alone (~1.27 us/pair in-context) handles all 64
    pairs in ~82 us/pass.  GPSIMD/ACT offload of pairs measured
    counterproductive (+0.6 us per offloaded pair, ng=0).
  - tail (~7 us): per-partition max (one 3D-AP reduce), global max via
    TensorE transpose + DVE max, -max broadcast with a diag + ones
    matmul, exp+rowsum on ScalarE, denominator via sumexp^T @ ones,
    reciprocal, per-(b,t)-row 1/D via a sel matmul, and the normalize
    folded into a single [64,128] scale AFTER the TensorE transpose,
    then one contiguous store.

Measured per-op in-context rates (isolated microbenches run ~2x slower
than in-pipeline => calibrate ops inside a DMA-paced pipeline, and
measure via pipelined unblocked call batches; serial per-call walls are
10 ms-quantized by the axon poll loop):
    DVE fp32 TT/AMR ~1.1-1.27 ns/elem,  ACT reduce ~1.6 ns/elem,
    GPSIMD TT ~4 ns/elem,  enc-stream DMA 428 GB/s.
NOTE: vector.tensor_tensor_reduce passes CoreSim but dies with a
runtime INTERNAL error on this axon terminal; affine_mul_reduce (custom
DVE uop) is the working fused multiply+reduce.
"""

import sys

if "/opt/trn_rl_repo" not in sys.path:
    sys.path.insert(0, "/opt/trn_rl_repo")

import numpy as np

import concourse.bass as bass
import concourse.mybir as mybir
import concourse.tile as tile
from concourse import bacc, bass_utils
from concourse.bass import ts
from concourse.bass_isa import ReduceOp
from concourse.masks import make_identity

S, B, H = 2048, 32, 1024
NCORES = 8
BS = B // NCORES  # 4 batches per core
P = 128
T = S // P  # 16 s-tiles per core
KC = H // P  # 8 contraction chunks
F32 = mybir.dt.float32
AX = mybir.AxisListType
ALU = mybir.AluOpType
ACT = mybir.ActivationFunctionType

ENC_BUFS = 6
ENC_BUFS2 = 8  # v2: DVE-bound stream; extra runway absorbs DMA jitter
PROD_BUFS = 3


def build_kernel_body(
    nc, tc, enc, hid_d, w_d, out_d, repeat=1, variant="full", et_alt=False,
    gps=False, batch2=False, w_ring="scalar", proto_in_loop=False,
):
    """Emit the per-core program.  enc/hid_d/w_d/out_d are DRAM APs.

    repeat > 1 re-runs the main streaming loop (for timing calibration
    only -- logits are simply overwritten, output stays correct).
    variant: "full" | "dmaonly" (skip compute) | "computeonly" (skip DMA),
    both for bottleneck isolation; their outputs are garbage."""
    import contextlib

    with contextlib.ExitStack() as ctx:
        consts = ctx.enter_context(tc.tile_pool(name="consts", bufs=1))
        wpool = ctx.enter_context(tc.tile_pool(name="wpool", bufs=1))
        epool = ctx.enter_context(
            tc.tile_pool(name="epool", bufs=3 if batch2 else ENC_BUFS)
        )
        vbpool = ctx.enter_context(tc.tile_pool(name="vbpool", bufs=1))
        prodpool = ctx.enter_context(tc.tile_pool(name="prodpool", bufs=PROD_BUFS))
        small = ctx.enter_context(tc.tile_pool(name="small", bufs=1))
        ptp = ctx.enter_context(tc.tile_pool(name="ptp", bufs=2, space="PSUM"))
        pvb = ctx.enter_context(tc.tile_pool(name="pvb", bufs=2, space="PSUM"))
        ptail = ctx.enter_context(tc.tile_pool(name="ptail", bufs=1, space="PSUM"))

        identity = consts.tile([P, P], F32)
        make_identity(nc, identity)
        ones = consts.tile([P, 1], F32)
        nc.gpsimd.memset(ones, 1.0)
        ones4 = consts.tile([BS, P], F32)
        nc.gpsimd.memset(ones4, 1.0)

        # Preload the exp activation table so the tail doesn't pay ~2.7us.
        warm = small.tile([1, 1], F32)
        nc.gpsimd.memset(warm, 1.0)
        nc.scalar.activation(warm, warm, ACT.Exp)

        # ---- prologue: hidden, W, and the broadcast v tiles ----
        # hid/W go on the ACT-issued HWDGE ring: ACT is idle during the
        # prologue (the +9us/pass interference seen when streaming et from
        # this ring does not apply here), and it frees the sync ring so the
        # enc stream starts at t=0 instead of behind 4 MB of W -- worth
        # ~14 us of single-call device time.
        w_view = w_d.rearrange("(kc kp) h -> kc kp h", kp=P)  # [8, 128, 1024]
        wdma = nc.scalar if w_ring == "scalar" else nc.sync
        vbcat = vbpool.tile([P, BS * H], F32)

        def emit_prologue():
            hid = small.tile([BS, H], F32, tag="hid", name="hid")
            wdma.dma_start(hid, hid_d)

            # hidT[:, 4*kc + b] = hidden[b, kc*128 : (kc+1)*128]
            hidT = small.tile([P, BS * KC], F32, tag="hidT", name="hidT")
            for kc in range(KC):
                pt = ptp.tile([P, BS], F32, tag="pt", name="pt")
                nc.tensor.transpose(pt, hid[:, ts(kc, P)], identity[0:BS, 0:BS])
                nc.scalar.copy(hidT[:, ts(kc, BS)], pt)

            wt = []
            for kc in range(KC):
                wtile = wpool.tile([P, H], F32, tag=f"w{kc}", name=f"w{kc}")
                wdma.dma_start(wtile, w_view[kc])
                wt.append(wtile)

            # v[b, :] = hidden[b] @ W, computed once (out partitions 0..3) ...
            v = small.tile([BS, H], F32, tag="v", name="v")
            for hh in range(2):
                accv = pvb.tile([BS, 512], F32, tag="acc", name="accv")
                for kc in range(KC):
                    nc.tensor.matmul(
                        accv,
                        lhsT=hidT[:, ts(kc, BS)],
                        rhs=wt[kc][:, ts(hh, 512)],
                        start=(kc == 0),
                        stop=(kc == KC - 1),
                    )
                nc.scalar.copy(v[:, ts(hh, 512)], accv)

            # ... then vbcat[p, b*H + h] = v[b, h] for every p: row b of v
            # replicated on all 128 partitions via a K=4 selector matmul,
            # lhsT[k, p] = identity[k, b] (= 1 iff k == b) broadcast along
            # free.  vbcat matches the et layout: ONE wide DVE mult per tile.
            for b in range(BS):
                for hh in range(2):
                    acc = pvb.tile([P, 512], F32, tag="acc", name="acc")
                    nc.tensor.matmul(
                        acc,
                        lhsT=identity[0:BS, b : b + 1].broadcast_to([BS, P]),
                        rhs=v[:, ts(hh, 512)],
                        start=True,
                        stop=True,
                    )
                    nc.scalar.copy(vbcat[:, ts(2 * b + hh, 512)], acc)

        if not proto_in_loop:
            emit_prologue()

        # ---- main loop: fused dot-products over the enc stream ----
        # s-tile t covers s in [128t, 128t+128), partition p <-> s = 128t + p.
        enc_view = enc.rearrange("(t p) b h -> t p (b h)", p=P)  # [16, 128, 4096]
        # batch2: one 4 MB DMA covers s-tiles 2u and 2u+1 (halves the DMA
        # issue/completion count; descriptor sizes are unchanged at 16 KB).
        enc_view2 = enc.rearrange("(u c p) b h -> u p c (b h)", c=2, p=P)
        logits = [
            small.tile([P, T], F32, tag=f"lg{b}", name=f"lg{b}") for b in range(BS)
        ]
        # DVE does the elementwise multiply; ScalarE reduces each product row
        # via activation(Copy, accum_out=...) -> 2-stage cross-engine pipeline.
        # ACT's throwaway output goes to PSUM (ScalarE is closer to PSUM).
        dumpp = ptail.tile([P, 1024], F32, tag="dumpp", name="dumpp")
        dump = small.tile([P, H], F32)
        if variant in ("dmaonly", "noact"):
            for b in range(BS):
                nc.vector.memset(logits[b], 0.0)
        cet = None
        if variant == "computeonly" or variant.startswith("mb_"):
            cet = epool.tile([P, BS * H], F32, tag="et", name="cet")
            nc.gpsimd.memset(cet, 0.001)
        if variant.startswith("mb_"):
            # --- microbenches: repeat emits ONLY the op under test ---
            for b in range(BS):
                nc.vector.memset(logits[b], 0.0)
            mb_prod = dump  # reuse the [P, H] scratch
            mb_wide = [
                epool.tile([P, BS * H], F32, tag="et", name=f"mbw{i}")
                for i in range(2)
            ]
            for _rep in range(repeat):
                if variant == "mb_ttr":  # 64 fused mult+reduce on DVE
                    mb_dummy = small.tile([P, 1], F32, tag="mbdummy")
                    for t in range(T):
                        for b in range(BS):
                            nc.vector.tensor_tensor_reduce(
                                mb_dummy.broadcast_to([P, H]),
                                cet[:, ts(b, H)],
                                vbcat[:, ts(b, H)],
                                scale=1.0,
                                scalar=0.0,
                                op0=ALU.mult,
                                op1=ALU.add,
                                accum_out=logits[b][:, t : t + 1],
                            )
                elif variant == "mb_gpsmul":  # 16 wide muls on GPSIMD
                    for t in range(T):
                        nc.gpsimd.tensor_mul(
                            mb_wide[t % 2], cet, vbcat[:, 0 : BS * H]
                        )
                elif variant == "mb_gpsmul1k":  # 64 1k-wide muls on GPSIMD
                    for t in range(T):
                        for b in range(BS):
                            nc.gpsimd.tensor_mul(
                                mb_prod, cet[:, ts(b, H)], vbcat[:, ts(b, H)]
                            )
                elif variant == "mb_actred_sb":  # 64 ACT reduces, SBUF dst
                    for t in range(T):
                        for b in range(BS):
                            nc.scalar.activation(
                                dump[:, 0:H],
                                cet[:, ts(b, H)],
                                ACT.Copy,
                                accum_out=logits[b][:, t : t + 1],
                            )
                elif variant == "mb_actred4k":  # 16 wide ACT (1 accum), SBUF
                    for t in range(T):
                        nc.scalar.activation(
                            mb_wide[t % 2],
                            cet,
                            ACT.Copy,
                            accum_out=logits[0][:, t : t + 1],
                        )
                elif variant == "mb_dvett2":  # 32 2k-wide DVE TT muls
                    for t in range(T):
                        for g in range(2):
                            nc.vector.tensor_mul(
                                mb_wide[t % 2][:, ts(g, 2 * H)],
                                cet[:, ts(g, 2 * H)],
                                vbcat[:, ts(g, 2 * H)],
                            )
                elif variant == "mb_actred":  # 64 1k-wide ACT accum-reduces
                    for t in range(T):
                        for b in range(BS):
                            nc.scalar.activation(
                                dumpp[:, 0:H],
                                cet[:, ts(b, H)],
                                ACT.Copy,
                                accum_out=logits[b][:, t : t + 1],
                            )
                elif variant == "mb_dvered":  # 64 1k-wide DVE reduces
                    for t in range(T):
                        for b in range(BS):
                            nc.vector.tensor_reduce(
                                logits[b][:, t : t + 1],
                                cet[:, ts(b, H)],
                                axis=AX.X,
                                op=ALU.add,
                            )
                elif variant == "mb_dvett":  # 16 4k-wide DVE TT muls
                    for t in range(T):
                        nc.vector.tensor_mul(
                            mb_wide[t % 2], cet, vbcat[:, 0 : BS * H]
                        )
                elif variant == "mb_ttr4k":  # 16 4k-wide TTR (one accum col)
                    mb_dummy4 = small.tile([P, 1], F32, tag="mbdummy4")
                    for t in range(T):
                        nc.vector.tensor_tensor_reduce(
                            mb_dummy4.broadcast_to([P, BS * H]),
                            cet,
                            vbcat[:, 0 : BS * H],
                            scale=1.0,
                            scalar=0.0,
                            op0=ALU.mult,
                            op1=ALU.add,
                            accum_out=logits[0][:, t : t + 1],
                        )
                else:
                    raise ValueError(variant)
        # width: full kernel reduces H elements per (t, b); halfwidth reduces
        # H/2 with identical instruction counts (overhead calibration only).
        wid = H // 2 if variant == "halfwidth" else H
        for _rep in range(repeat):
            if proto_in_loop:
                emit_prologue()
            et2 = None
            for t in range(T):
                if variant == "computeonly":
                    et = cet
                elif batch2:
                    if t % 2 == 0:
                        et2 = epool.tile(
                            [P, 2 * BS * H], F32, tag="et", name="et2"
                        )
                        nc.sync.dma_start(et2, enc_view2[t // 2])
                    et = et2[:, ts(t % 2, BS * H)]
                else:
                    et = epool.tile([P, BS * wid], F32, tag="et", name="et")
                    eng = nc.scalar if (et_alt and t % 2 == 1) else nc.sync
                    if variant == "halfwidth":
                        eng.dma_start(et, enc_view[t][:, 0 : BS * wid])
                    else:
                        eng.dma_start(et, enc_view[t])
                if variant == "dmaonly":
                    # touch one column so DCE keeps the DMA
                    nc.vector.tensor_scalar_mul(dump[:, 0:1], et[:, 0:1], 1.0)
                    continue
                if variant == "nodve":
                    for b in range(BS):
                        # ACT accumulates straight from et (no multiply)
                        nc.scalar.activation(
                            dump,
                            et[:, ts(b, H)],
                            ACT.Copy,
                            accum_out=logits[b][:, t : t + 1],
                        )
                    continue
                prod = prodpool.tile([P, BS * wid], F32, tag="prod", name="prod")
                # optionally offload some multiplies to the (otherwise idle)
                # GPSIMD engine to unload DVE
                if gps and t % 5 == 2:
                    nc.gpsimd.tensor_mul(prod, et, vbcat[:, 0 : BS * wid])
                else:
                    nc.vector.tensor_mul(prod, et, vbcat[:, 0 : BS * wid])
                if variant == "noact":
                    continue
                for b in range(BS):
                    # ACT does most reductions; hand a few to DVE to balance
                    # the two engines (both sit just above the DMA floor).
                    dve_red = (b == 3 and t % 2 == 1) if gps else (
                        b == 3 and t % 4 == 1
                    )
                    if dve_red:
                        nc.vector.tensor_reduce(
                            logits[b][:, t : t + 1],
                            prod[:, ts(b, wid)],
                            axis=AX.X,
                            op=ALU.add,
                        )
                    else:
                        nc.scalar.activation(
                            dumpp[:, 0:wid],
                            prod[:, ts(b, wid)],
                            ACT.Copy,
                            accum_out=logits[b][:, t : t + 1],
                        )

        # ---- tail: softmax over s (2048 values per b), two-pass exp ----
        # pass 1 computes exp(x - M1) and its sums (M1 = per-partition max,
        # shifted enough for fp range); pass 2 re-exps with the exact bias
        # -M1b - ln(sum exp(x - M1b)), which folds the normalization in.
        m1 = small.tile([P, BS], F32)
        for b in range(BS):
            nc.vector.tensor_reduce(
                m1[:, b : b + 1], logits[b], axis=AX.X, op=ALU.max
            )
        # global per-b max via TensorE transpose + free-dim reduce
        pm1t = ptp.tile([BS, P], F32, tag="pt", name="pm1t")
        nc.tensor.transpose(pm1t, m1, identity)
        mx4 = small.tile([BS, 1], F32)
        nc.vector.tensor_reduce(mx4, pm1t, axis=AX.X, op=ALU.max)
        # broadcast -mx4 to all partitions: diag(-mx4) then ones^T @ diag
        dg4 = small.tile([BS, BS], F32)
        nc.vector.tensor_scalar(
            dg4,
            identity[0:BS, 0:BS],
            mx4,
            -1.0,
            op0=ALU.mult,
            op1=ALU.mult,
        )
        pneg = ptp.tile([P, BS], F32, tag="pt", name="pneg")
        nc.tensor.matmul(pneg, lhsT=ones4, rhs=dg4, start=True, stop=True)
        negmax = small.tile([P, BS], F32)
        nc.scalar.copy(negmax, pneg)

        probs = small.tile([P, BS * T], F32)  # [128, 64], col = b*16 + t
        sumexp = small.tile([P, BS], F32)
        for b in range(BS):
            nc.scalar.activation(
                probs[:, ts(b, T)],
                logits[b],
                ACT.Exp,
                bias=negmax[:, b : b + 1],
                scale=1.0,
                accum_out=sumexp[:, b : b + 1],
            )

        # denominator, directly transposed: den[b] = sum_p sumexp[p, b]
        # via lhsT=sumexp, rhs=ones -> [4, 1]; reciprocal on DVE, then
        # broadcast 1/den to all partitions with the diag + ones matmul
        # and scale probs per b.
        pdent = ptp.tile([BS, 1], F32, tag="pt", name="pdent")
        nc.tensor.matmul(pdent, lhsT=sumexp, rhs=ones, start=True, stop=True)
        rdent = small.tile([BS, 1], F32)
        nc.vector.reciprocal(rdent, pdent)
        dg4b = small.tile([BS, BS], F32)
        nc.vector.tensor_scalar(
            dg4b,
            identity[0:BS, 0:BS],
            rdent,
            None,
            op0=ALU.mult,
        )
        pb2 = ptp.tile([P, BS], F32, tag="pt", name="pb2")
        nc.tensor.matmul(pb2, lhsT=ones4, rhs=dg4b, start=True, stop=True)
        rbc = small.tile([P, BS], F32)
        nc.scalar.copy(rbc, pb2)
        for b in range(BS):
            nc.vector.tensor_scalar_mul(
                probs[:, ts(b, T)], probs[:, ts(b, T)], rbc[:, b : b + 1]
            )

        # ---- store: transpose so DRAM rows are contiguous ----
        # probs[p, b*16+t] -> oT[b*16+t, p];  out[b, 128t + p] = oT[(b,t), p]
        poT = ptail.tile([BS * T, P], F32, tag="poT")
        nc.tensor.transpose(poT, probs, identity)
        oT = small.tile([BS * T, P], F32)
        nc.scalar.copy(oT, poT)
        out_view = out_d.rearrange("b (t p) -> (b t) p", p=P)  # [64, 128]
        nc.sync.dma_start(out_view, oT)


def build_nc(repeat=1, variant="full", et_alt=False, gps=False, batch2=False,
             w_ring="scalar", proto_in_loop=False, timing=False):
    nc = bacc.Bacc(
        "TRN2",
        target_bir_lowering=False,
        debug=False,
        num_devices=NCORES,
    )
    # timing=True: enc/W become Internal scratch (garbage values, same
    # addresses + DMA traffic) so the 268 MB axon input transfer per call
    # disappears and wall-clock repeat-deltas get ~10x less noise.
    big_kind = "Internal" if timing else "ExternalInput"
    enc = nc.dram_tensor("enc", [S, BS, H], F32, kind=big_kind).ap()
    hid_d = nc.dram_tensor("hidden", [BS, H], F32, kind="ExternalInput").ap()
    w_d = nc.dram_tensor("w", [H, H], F32, kind=big_kind).ap()
    out_d = nc.dram_tensor("out", [BS, S], F32, kind="ExternalOutput").ap()
    with tile.TileContext(nc) as tc:
        build_kernel_body(
            nc, tc, enc, hid_d, w_d, out_d, repeat=repeat, variant=variant,
            et_alt=et_alt, gps=gps, batch2=batch2, w_ring=w_ring,
            proto_in_loop=proto_in_loop,
        )
    nc.compile()
    return nc


def make_in_maps(hidden, encoder_outputs, W):
    hidden = np.asarray(hidden, dtype=np.float32)
    encoder_outputs = np.asarray(encoder_outputs, dtype=np.float32)
    W = np.ascontiguousarray(np.asarray(W, dtype=np.float32))
    in_maps = []
    for c in range(NCORES):
        in_maps.append(
            {
                "enc": np.ascontiguousarray(
                    encoder_outputs[:, c * BS : (c + 1) * BS, :]
                ),
                "hidden": np.ascontiguousarray(hidden[c * BS : (c + 1) * BS, :]),
                "w": W,
            }
        )
    return in_maps


_NC_CACHE = {}

USE_HOSTU = False  # device computes u = hidden @ W (W read on-device)


def get_nc():
    if "nc" not in _NC_CACHE:
        _NC_CACHE["nc"] = build_nc2(hostu=USE_HOSTU)
    return _NC_CACHE["nc"]


def kernel(hidden, encoder_outputs, W, b, **_unused):
    # The linear-layer bias contributes hidden[b].bias to every logit of
    # row b, a per-row constant that cancels in the softmax -> unused.
    nc = get_nc()
    in_maps = make_in_maps2(hidden, encoder_outputs, W, hostu=USE_HOSTU)
    res = bass_utils.run_bass_kernel_spmd(
        nc, in_maps, core_ids=list(range(NCORES))
    )
    outs = [res.results[c]["out"] for c in range(NCORES)]
    full = np.concatenate(outs, axis=0)  # [32, 2048]
    return full[None, :, :].astype(np.float32, copy=False)


# ===================== v2: fused AMR stream =====================
# Stream compute per (t, b) pair is ONE DVE affine_mul_reduce:
#   dump = (et_seg * 1 + 0) * vbcat_seg ; logits_col = sum(dump)
# -- one pass over the data instead of multiply + separate reduce, so DVE
# alone (~1.2 us/pair in-context) covers all 64 pairs under the 78 us DMA
# floor; `ng` pairs (b=3 on evenly spaced tiles) go GPSIMD-mul + ACT-reduce
# to give DVE slack.  W is loaded FIRST on the sync ring so vbcat is ready
# ~14 us in and the et stream (queued behind it) never stalls compute.

GPS_NG = 0  # GPSIMD offload measured counterproductive (~+0.6 us/pair)


def build_kernel_body2(
    nc, tc, enc, hid_d, w_d, out_d, repeat=1, ng=GPS_NG, u_d=None,
    variant="full", ebufs=ENC_BUFS2,
):
    import contextlib

    with contextlib.ExitStack() as ctx:
        consts = ctx.enter_context(tc.tile_pool(name="consts", bufs=1))
        wpool = ctx.enter_context(tc.tile_pool(name="wpool", bufs=1))
        epool = ctx.enter_context(tc.tile_pool(name="epool", bufs=ebufs))
        vbpool = ctx.enter_context(tc.tile_pool(name="vbpool", bufs=1))
        gprod = ctx.enter_context(tc.tile_pool(name="gprod", bufs=2))
        small = ctx.enter_context(tc.tile_pool(name="small", bufs=1))
        ptp = ctx.enter_context(tc.tile_pool(name="ptp", bufs=1, space="PSUM"))
        pvb = ctx.enter_context(tc.tile_pool(name="pvb", bufs=2, space="PSUM"))
        ptail = ctx.enter_context(tc.tile_pool(name="ptail", bufs=1, space="PSUM"))

        identity = consts.tile([P, P], F32)
        make_identity(nc, identity)
        ones = consts.tile([P, 1], F32)
        nc.gpsimd.memset(ones, 1.0)
        ones4 = consts.tile([BS, P], F32)
        nc.gpsimd.memset(ones4, 1.0)
        warm = small.tile([1, 1], F32)
        nc.gpsimd.memset(warm, 1.0)
        nc.scalar.activation(warm, warm, ACT.Exp)

        # sel[k, m] = 1 iff m // T == k: iota gives m - T*k per row, then
        # two compares AND'd by multiply.  (3D broadcast lhsT APs fail the
        # BIR matmul verifier, so the mask is materialized.)
        sel = consts.tile([BS, BS * T], F32)
        selio = consts.tile([BS, BS * T], F32)
        nc.gpsimd.iota(
            selio,
            pattern=[[1, BS * T]],
            channel_multiplier=-T,
            allow_small_or_imprecise_dtypes=True,
        )
        sel_lo = consts.tile([BS, BS * T], F32)
        sel_hi = consts.tile([BS, BS * T], F32)
        nc.vector.tensor_scalar(sel_lo, selio, 0.0, None, op0=ALU.is_ge)
        nc.vector.tensor_scalar(sel_hi, selio, float(T), None, op0=ALU.is_lt)
        nc.vector.tensor_tensor(sel, sel_lo, sel_hi, op=ALU.mult)

        vbcat = vbpool.tile([P, BS * H], F32)

        # ---- prologue ----
        def build_vbcat_from(v):
            # vbcat[p, b*H+h] = v[b, h] on all partitions via selector matmuls
            for b in range(BS):
                for hh in range(2):
                    acc = pvb.tile([P, 512], F32, tag="acc", name="acc")
                    nc.tensor.matmul(
                        acc,
                        lhsT=identity[0:BS, b : b + 1].broadcast_to([BS, P]),
                        rhs=v[:, ts(hh, 512)],
                        start=True,
                        stop=True,
                    )
                    nc.scalar.copy(vbcat[:, ts(2 * b + hh, 512)], acc)

        if u_d is not None:
            u = small.tile([BS, H], F32, tag="u", name="u")
            nc.sync.dma_start(u, u_d)
            build_vbcat_from(u)
        else:
            # W first and ALONE on the sync ring: compute is gated on the
            # full W, so it must finish before anything else; the et stream
            # queues behind it on the same ring.  (Splitting W across both
            # rings lets et DMAs interleave and steal bandwidth, delaying
            # vbcat and the compute start by ~15 us.)
            hid = small.tile([BS, H], F32, tag="hid", name="hid")
            nc.scalar.dma_start(hid, hid_d)
            wt = []
            w_view = w_d.rearrange("(kc kp) h -> kc kp h", kp=P)
            for kc in range(KC):
                wtile = wpool.tile([P, H], F32, tag=f"w{kc}", name=f"w{kc}")
                nc.sync.dma_start(wtile, w_view[kc])
                wt.append(wtile)
            hidT = small.tile([P, BS * KC], F32, tag="hidT", name="hidT")
            for kc in range(KC):
                pt = ptp.tile([P, BS], F32, tag="pt", name="pt")
                nc.tensor.transpose(pt, hid[:, ts(kc, P)], identity[0:BS, 0:BS])
                nc.scalar.copy(hidT[:, ts(kc, BS)], pt)
            # Keep PE busy through the ~12 us W load: the HAM clock gate
            # needs ~3.4 us of recent activity, and a cold PE runs the v /
            # vbcat matmuls at half clock (853 ns vs ~430 ns each).  Dummy
            # matmuls (no data deps) warm it so the post-W chain is short.
            wu = pvb.tile([P, 128], F32, tag="wu", name="wu")
            for _ in range(24):
                nc.tensor.matmul(
                    wu, lhsT=identity, rhs=identity, start=True, stop=True
                )
            v = small.tile([BS, H], F32, tag="v", name="v")
            for hh in range(2):
                accv = pvb.tile([BS, 512], F32, tag="acc", name="accv")
                for kc in range(KC):
                    nc.tensor.matmul(
                        accv,
                        lhsT=hidT[:, ts(kc, BS)],
                        rhs=wt[kc][:, ts(hh, 512)],
                        start=(kc == 0),
                        stop=(kc == KC - 1),
                    )
                nc.scalar.copy(v[:, ts(hh, 512)], accv)
            build_vbcat_from(v)

        # ---- stream ----
        enc_view = enc.rearrange("(t p) b h -> t p (b h)", p=P)
        logits_all = small.tile([P, BS * T], F32, name="logits")  # col b*T+t
        dump = small.tile([P, H], F32, name="dump")  # AMR throwaway product
        dump2 = small.tile([P, H], F32, name="dump2")  # ACT throwaway (own
        # tile: sharing dump with the DVE AMRs would add cross-engine WAW
        # sems serializing the two engines)
        gps_set = set()
        if ng > 0:
            gps_set = {min(T - 1, round(i * T / ng)) for i in range(ng)}
        cet = None
        if variant == "computeonly":
            cet = epool.tile([P, BS * H], F32, tag="et", name="cet")
            nc.gpsimd.memset(cet, 0.001)
        for _rep in range(repeat):
            for t in range(T):
                if variant == "computeonly":
                    et = cet
                else:
                    et = epool.tile([P, BS * H], F32, tag="et", name="et")
                    nc.sync.dma_start(et, enc_view[t])
                if variant == "dmaonly":
                    nc.vector.tensor_scalar_mul(dump[:, 0:1], et[:, 0:1], 1.0)
                    continue
                for b in range(BS):
                    col = logits_all[:, b * T + t : b * T + t + 1]
                    if b == 3 and t in gps_set:
                        pg = gprod.tile([P, H], F32, tag="pg", name="pg")
                        nc.gpsimd.tensor_mul(
                            pg, et[:, ts(b, H)], vbcat[:, ts(b, H)]
                        )
                        nc.scalar.activation(
                            dump2, pg, ACT.Copy, accum_out=col
                        )
                    else:
                        nc.vector.affine_mul_reduce(
                            dump,
                            col,
                            et[:, ts(b, H)],
                            vbcat[:, ts(b, H)],
                            scale=1.0,
                            bias=0.0,
                        )

        # ---- tail: softmax over s per b ----
        if variant == "dmaonly":
            nc.vector.memset(logits_all, 0.0)
        lg3 = logits_all.rearrange("p (b t) -> p b t", b=BS)
        m1 = small.tile([P, BS], F32, name="m1")
        nc.vector.tensor_reduce(m1, lg3, axis=AX.X, op=ALU.max)
        pm1t = ptp.tile([BS, P], F32, tag="pt", name="pm1t")
        nc.tensor.transpose(pm1t, m1, identity)
        mx4 = small.tile([BS, 1], F32, name="mx4")
        nc.vector.tensor_reduce(mx4, pm1t, axis=AX.X, op=ALU.max)
        dg4 = small.tile([BS, BS], F32, name="dg4")
        nc.vector.tensor_scalar(
            dg4, identity[0:BS, 0:BS], mx4, -1.0, op0=ALU.mult, op1=ALU.mult
        )
        pneg = ptp.tile([P, BS], F32, tag="pt", name="pneg")
        nc.tensor.matmul(pneg, lhsT=ones4, rhs=dg4, start=True, stop=True)
        negmax = small.tile([P, BS], F32, name="negmax")
        nc.scalar.copy(negmax, pneg)  # ACT bias must be SBUF

        probs = small.tile([P, BS * T], F32, name="probs")
        sumexp = small.tile([P, BS], F32, name="sumexp")
        for b in range(BS):
            nc.scalar.activation(
                probs[:, ts(b, T)],
                logits_all[:, ts(b, T)],
                ACT.Exp,
                bias=negmax[:, b : b + 1],
                scale=1.0,
                accum_out=sumexp[:, b : b + 1],
            )

        pdent = ptail.tile([BS, 1], F32, tag="pdent", name="pdent")
        nc.tensor.matmul(pdent, lhsT=sumexp, rhs=ones, start=True, stop=True)
        poT = ptail.tile([BS * T, P], F32, tag="poT", name="poT")
        nc.tensor.transpose(poT, probs, identity)
        rdent = small.tile([BS, 1], F32, name="rdent")
        nc.vector.reciprocal(rdent, pdent)
        prep = ptail.tile([BS * T, 1], F32, tag="prep", name="prep")
        nc.tensor.matmul(prep, lhsT=sel, rhs=rdent, start=True, stop=True)
        oT = small.tile([BS * T, P], F32, name="oT")
        nc.vector.tensor_scalar_mul(oT, poT, prep)
        out_view = out_d.rearrange("b (t p) -> (b t) p", p=P)
        nc.sync.dma_start(out_view, oT)


def build_nc2(repeat=1, ng=GPS_NG, hostu=False, timing=False, variant="full",
              ebufs=ENC_BUFS2):
    nc = bacc.Bacc(
        "TRN2", target_bir_lowering=False, debug=False, num_devices=NCORES
    )
    big_kind = "Internal" if timing else "ExternalInput"
    enc = nc.dram_tensor("enc", [S, BS, H], F32, kind=big_kind).ap()
    hid_d = nc.dram_tensor("hidden", [BS, H], F32, kind="ExternalInput").ap()
    u_d = None
    w_d = None
    if hostu:
        u_d = nc.dram_tensor("u", [BS, H], F32, kind="ExternalInput").ap()
    else:
        w_d = nc.dram_tensor("w", [H, H], F32, kind=big_kind).ap()
    out_d = nc.dram_tensor("out", [BS, S], F32, kind="ExternalOutput").ap()
    with tile.TileContext(nc) as tc:
        build_kernel_body2(
            nc, tc, enc, hid_d, w_d, out_d, repeat=repeat, ng=ng, u_d=u_d,
            variant=variant, ebufs=ebufs,
        )
    nc.compile()
    return nc


def make_in_maps2(hidden, encoder_outputs, W, hostu=False):
    hidden = np.asarray(hidden, dtype=np.float32)
    encoder_outputs = np.asarray(encoder_outputs, dtype=np.float32)
    W = np.ascontiguousarray(np.asarray(W, dtype=np.float32))
    in_maps = []
    for c in range(NCORES):
        m = {
            "enc": np.ascontiguousarray(
                encoder_outputs[:, c * BS : (c + 1) * BS, :]
            ),
            "hidden": np.ascontiguousarray(hidden[c * BS : (c + 1) * BS, :]),
        }
        if hostu:
            m["u"] = np.ascontiguousarray(m["hidden"] @ W)
        else:
            m["w"] = W
        in_maps.append(m)
    return in_maps


def build_nc_empty():
    """Minimal kernel: memset + store the output. Measures per-call launch
    overhead so (variant - empty) isolates device kernel time."""
    nc = bacc.Bacc(
        "TRN2", target_bir_lowering=False, debug=False, num_devices=NCORES
    )
    hid_d = nc.dram_tensor("hidden", [BS, H], F32, kind="ExternalInput").ap()
    out_d = nc.dram_tensor("out", [BS, S], F32, kind="ExternalOutput").ap()
    with tile.TileContext(nc) as tc:
        with tc.tile_pool(name="p", bufs=1) as pool:
            o = pool.tile([BS, S], F32)
            nc.vector.memset(o, 0.0)
            nc.sync.dma_start(out_d, o)
    nc.compile()
    return nc

